# revision 3
# baseline (speedup 1.0000x reference)
"""Conv3dFFT Trainium2 kernel.

Strategy (8 NeuronCores, sharded over Cin: 4 channels/core, all 16 Cout):
  out[co,ci] = decimate(crop(irfftn(rfftn(pad(x[ci])) * rfftn(w[co,ci]))))
  == circular 3D convolution of padded x with w, sampled at odd coords 1..61.

Per core, per packet (2 couts coA,coB x 1 cin): pack v = wA + i*wB, run a
full complex 3D DFT pipeline via PE matmuls (separable, one axis per stage,
DFT matrices stationary), pointwise-multiply by host-precomputed Xf, inverse
transform truncated to the 31 odd output points per axis. Re/Im of the final
inverse give coA/coB results.  The pointwise complex-multiply is folded into
the first inverse stage's PSUM accumulation (U_A = V*Re(Xf), U_B = V*Im(Xf),
then two accumulating matmuls with sign-combined stationaries).

Layouts per stage keep the contraction axis in the 128-partition dim
([re(0:64) | im(64:128)] stacking); the partition<->free axis rotations
between stages are SBUF->SBUF DMAs (64 per rotation, one per bin column).

Scales: fwd stationaries x1/8 each (3 stages), Xf x1/512 => (1/8)^3/512 =
1/64^3 = exact inverse-DFT normalization; inverse stationaries unscaled.
Intermediates ride in fp16 (sigma ~ 1); matmuls fp32r (S1) / fp16 (rest),
accumulation always fp32 in PSUM.
"""

import numpy as np

N = 64
NS = 31
W0 = 2.0 * np.pi / N
S_GRID = 2 * np.arange(NS) + 1
NCORES = 8
NCI = 4   # cin channels per core
NPR = 8   # cout-pair packets per cin

_CACHE = {}


def _round_f32r(x):
    u = np.ascontiguousarray(x, dtype=np.float32).view(np.uint32)
    r = ((u.astype(np.uint64) + 0x800) & 0xFFFFF000).astype(np.uint32)
    return r.view(np.float32)


def _fwd_stationary(scale=0.125):
    t = np.arange(N)
    C = np.cos(W0 * np.outer(t, t)) * scale
    S = np.sin(W0 * np.outer(t, t)) * scale
    F = np.zeros((128, 128))
    F[0:64, 0:64] = C
    F[64:128, 0:64] = S
    F[0:64, 64:128] = -S
    F[64:128, 64:128] = C
    return F


def _inv_pair_stationary():
    kz = np.arange(N)
    Gr = np.cos(W0 * np.outer(kz, S_GRID))
    Gi = np.sin(W0 * np.outer(kz, S_GRID))
    G4A = np.zeros((128, 62))
    G4A[0:64, 0:31] = Gr
    G4A[64:128, 0:31] = -Gi
    G4A[0:64, 31:62] = Gi
    G4A[64:128, 31:62] = Gr
    G4B = np.zeros((128, 62))
    G4B[0:64, 0:31] = -Gi
    G4B[64:128, 0:31] = -Gr
    G4B[0:64, 31:62] = Gr
    G4B[64:128, 31:62] = -Gi
    return G4A, G4B


def _inv_stationary():
    k = np.arange(N)
    Gr = np.cos(W0 * np.outer(k, S_GRID))
    Gi = np.sin(W0 * np.outer(k, S_GRID))
    G = np.zeros((128, 62))
    G[0:64, 0:31] = Gr
    G[64:128, 0:31] = -Gi
    G[0:64, 31:62] = Gi
    G[64:128, 31:62] = Gr
    return G


def _build_nc(nci=NCI, npr=NPR, xp_bufs=2):
    import concourse.mybir as mybir
    import concourse.tile as tile
    from concourse import bacc

    f32 = mybir.dt.float32
    f32r = mybir.dt.float32r
    f16 = mybir.dt.float16

    nc = bacc.Bacc(None, target_bir_lowering=False)

    w_ext = nc.dram_tensor("w", [nci, npr, 128, 64, 64], f32r, kind="ExternalInput")
    xr_ext = nc.dram_tensor("xr", [nci, 128, 64, 64], f16, kind="ExternalInput")
    xi_ext = nc.dram_tensor("xi", [nci, 128, 64, 64], f16, kind="ExternalInput")
    o_ext = nc.dram_tensor("o", [nci, npr, 62, 961], f32, kind="ExternalOutput")

    F1_h = _round_f32r(_fwd_stationary())
    Ff_h = _fwd_stationary().astype(np.float16)
    G4A_h, G4B_h = (m.astype(np.float16) for m in _inv_pair_stationary())
    Gi_h = _inv_stationary().astype(np.float16)

    F1_d = nc.inline_tensor(F1_h, name="F1")
    Ff_d = nc.inline_tensor(Ff_h, name="Ffwd")
    G4A_d = nc.inline_tensor(np.ascontiguousarray(G4A_h), name="G4A")
    G4B_d = nc.inline_tensor(np.ascontiguousarray(G4B_h), name="G4B")
    Gi_d = nc.inline_tensor(Gi_h, name="Ginv")

    with tile.TileContext(nc) as tc:
        with (
            tc.tile_pool(name="const", bufs=1) as const,
            tc.tile_pool(name="wp", bufs=2) as wp,
            tc.tile_pool(name="xp", bufs=xp_bufs) as xp,
            tc.tile_pool(name="leg1", bufs=2) as leg1,
            tc.tile_pool(name="leg2", bufs=2) as leg2,
            tc.tile_pool(name="leg3", bufs=2) as leg3,
            tc.tile_pool(name="leg4", bufs=2) as leg4,
            tc.tile_pool(name="up", bufs=3) as up,
            tc.tile_pool(name="op", bufs=2) as op,
            tc.tile_pool(name="psA", bufs=2, space="PSUM") as psA,
            tc.tile_pool(name="psB", bufs=2, space="PSUM") as psB,
        ):
            F1 = const.tile([128, 128], f32r)
            nc.sync.dma_start(F1[:], F1_d[:].bitcast(f32r))
            Ff = const.tile([128, 128], f16)
            nc.sync.dma_start(Ff[:], Ff_d[:])
            G4A = const.tile([128, 62], f16)
            nc.sync.dma_start(G4A[:], G4A_d[:])
            G4B = const.tile([128, 62], f16)
            nc.sync.dma_start(G4B[:], G4B_d[:])
            Gi = const.tile([128, 62], f16)
            nc.sync.dma_start(Gi[:], Gi_d[:])

            for cl in range(nci):
                XR = xp.tile([128, 64, 64], f16, tag="XR")
                XI = xp.tile([128, 64, 64], f16, tag="XI")
                nc.sync.dma_start(XR[:], xr_ext[cl])
                nc.sync.dma_start(XI[:], xi_ext[cl])
                XRf = XR[:].rearrange("p a b -> p (a b)")
                XIf = XI[:].rearrange("p a b -> p (a b)")

                for pr in range(npr):
                    w_sb = wp.tile([128, 64, 64], f32r, tag="w")
                    nc.sync.dma_start(w_sb[:], w_ext[cl, pr])
                    wf = w_sb[:].rearrange("p a b -> p (a b)")

                    # ---- S1: x-DFT (fp32r). psum [128,1024] x4; evac -> A1 fp16
                    A1 = leg1.tile([128, 64, 64], f16, tag="A1")
                    A1f = A1[:].rearrange("p a b -> p (a b)")
                    for t in range(4):
                        ps = psA.tile([128, 1024], f32, tag="psA")
                        nc.tensor.matmul(ps[:, 0:512], F1[:], wf[:, 1024 * t:1024 * t + 512],
                                         start=True, stop=True)
                        nc.tensor.matmul(ps[:, 512:1024], F1[:], wf[:, 1024 * t + 512:1024 * t + 1024],
                                         start=True, stop=True)
                        nc.scalar.copy(A1f[:, 1024 * t:1024 * (t + 1)], ps[:])

                    # ---- T1: rotate kx<->y  B1[y+64h, kx, z] = A1[kx+64h, y, z]
                    B1 = leg2.tile([128, 64, 64], f16, tag="B1")
                    for k in range(64):
                        nc.sync.dma_start(B1[:, k, :], A1[k::64, :, :])

                    # ---- S2: y-DFT (fp16), stream (z-outer, kx-inner)
                    B1v = B1[:].rearrange("p kx z -> p z kx")
                    A2 = leg1.tile([128, 64, 64], f16, tag="A2")  # [ky|h, z, kx]
                    A2f = A2[:].rearrange("p a b -> p (a b)")
                    for t in range(4):
                        ps = psA.tile([128, 1024], f32, tag="psA")
                        nc.tensor.matmul(ps[:, 0:512], Ff[:], B1v[:, 16 * t:16 * t + 8, :],
                                         start=True, stop=True)
                        nc.tensor.matmul(ps[:, 512:1024], Ff[:], B1v[:, 16 * t + 8:16 * t + 16, :],
                                         start=True, stop=True)
                        nc.scalar.copy(A2f[:, 1024 * t:1024 * (t + 1)], ps[:])

                    # ---- T2: rotate ky<->z  B2[z+64h, ky, kx] = A2[ky+64h, z, kx]
                    B2 = leg2.tile([128, 64, 64], f16, tag="B2")
                    for k in range(64):
                        nc.sync.dma_start(B2[:, k, :], A2[k::64, :, :])

                    # ---- S3: z-DFT (fp16) + PW muls + S4 accumulate (inv-z)
                    A4 = leg3.tile([62, 64, 64], f16, tag="A4")  # [sz|h, ky, kx]
                    A4f = A4[:].rearrange("p a b -> p (a b)")
                    for t in range(4):
                        ps = psA.tile([128, 1024], f32, tag="psA")
                        nc.tensor.matmul(ps[:, 0:512], Ff[:], B2[:, 16 * t:16 * t + 8, :],
                                         start=True, stop=True)
                        nc.tensor.matmul(ps[:, 512:1024], Ff[:], B2[:, 16 * t + 8:16 * t + 16, :],
                                         start=True, stop=True)
                        UA = up.tile([128, 1024], f16, tag="UA")
                        UB = up.tile([128, 1024], f16, tag="UB")
                        nc.vector.tensor_mul(UA[:], ps[:], XRf[:, 1024 * t:1024 * (t + 1)])
                        nc.vector.tensor_mul(UB[:], ps[:], XIf[:, 1024 * t:1024 * (t + 1)])
                        ps4 = psB.tile([62, 1024], f32, tag="psB")
                        for c in range(2):
                            sl = slice(512 * c, 512 * (c + 1))
                            nc.tensor.matmul(ps4[:, sl], G4A[:], UA[:, sl], start=True, stop=False)
                            nc.tensor.matmul(ps4[:, sl], G4B[:], UB[:, sl], start=False, stop=True)
                        nc.scalar.copy(A4f[:, 1024 * t:1024 * (t + 1)], ps4[:])

                    # ---- T3: rotate ky<->sz  B4[ky+64h, sz, kx] = A4[sz+31h, ky, kx]
                    B4 = leg4.tile([128, 31, 64], f16, tag="B4")
                    for k in range(64):
                        nc.sync.dma_start(B4[k::64, :, :], A4[:, k, :])

                    # ---- S5: inv-y (fp16), stream (kx-outer, sz-inner)
                    B4v = B4[:].rearrange("p sz kx -> p kx sz")
                    A5 = leg3.tile([62, 64, 31], f16, tag="A5")  # [sy|h, kx, sz]
                    A5f = A5[:].rearrange("p a b -> p (a b)")
                    for t in range(2):
                        ps5 = psB.tile([62, 1024], f32, tag="psB")
                        nc.tensor.matmul(ps5[:, 0:496], Gi[:], B4v[:, 32 * t:32 * t + 16, :],
                                         start=True, stop=True)
                        nc.tensor.matmul(ps5[:, 512:1008], Gi[:], B4v[:, 32 * t + 16:32 * t + 32, :],
                                         start=True, stop=True)
                        nc.vector.tensor_copy(A5f[:, 992 * t:992 * t + 496], ps5[:, 0:496])
                        nc.vector.tensor_copy(A5f[:, 992 * t + 496:992 * (t + 1)], ps5[:, 512:1008])

                    # ---- T4: rotate kx<->sy  B5[kx+64h, sy, sz] = A5[sy+31h, kx, sz]
                    B5 = leg4.tile([128, 31, 31], f16, tag="B5")
                    for k in range(64):
                        nc.sync.dma_start(B5[k::64, :, :], A5[:, k, :])

                    # ---- S6: inv-x -> [outA 31 | outB 31, (sy, sz)]
                    B5f = B5[:].rearrange("p a b -> p (a b)")
                    ps6 = psB.tile([62, 1024], f32, tag="psB")
                    nc.tensor.matmul(ps6[:, 0:512], Gi[:], B5f[:, 0:512], start=True, stop=True)
                    nc.tensor.matmul(ps6[:, 512:961], Gi[:], B5f[:, 512:961], start=True, stop=True)
                    o_sb = op.tile([62, 961], f32, tag="o")
                    nc.vector.tensor_copy(o_sb[:], ps6[:, 0:961])
                    nc.sync.dma_start(o_ext[cl, pr], o_sb[:])

    nc.compile()
    return nc


def _get_nc(nci=NCI, npr=NPR):
    key = (nci, npr)
    if key not in _CACHE:
        _CACHE[key] = _build_nc(nci, npr)
    return _CACHE[key]


def _host_prep(x, weight, nci=NCI, npr=NPR):
    """Build per-core input maps."""
    x = np.asarray(x, dtype=np.float32)
    w = np.asarray(weight, dtype=np.float32)

    xp_ = np.zeros((32, 64, 64, 64), dtype=np.float64)
    xp_[:, 1:63, 1:63, 1:63] = x[0]
    Xf = np.fft.fftn(xp_, axes=(1, 2, 3)) / 512.0  # [ci, kx, ky, kz]

    in_maps = []
    for c in range(NCORES):
        ci0 = c * NCI
        wdev = np.empty((nci, npr, 128, 64, 64), dtype=np.float32)
        xr = np.empty((nci, 128, 64, 64), dtype=np.float16)
        xi = np.empty((nci, 128, 64, 64), dtype=np.float16)
        for cl in range(nci):
            ci = ci0 + cl
            for pr in range(npr):
                wdev[cl, pr, 0:64] = w[2 * pr, ci]
                wdev[cl, pr, 64:128] = w[2 * pr + 1, ci]
            Xt = Xf[ci].transpose(2, 1, 0)  # [kz, ky, kx]
            xr[cl, 0:64] = Xt.real
            xr[cl, 64:128] = Xt.real
            xi[cl, 0:64] = Xt.imag
            xi[cl, 64:128] = Xt.imag
        in_maps.append({"w": wdev, "xr": xr, "xi": xi})
    return in_maps


def kernel(x, weight):
    from concourse.bass_utils import run_bass_kernel_spmd

    nc = _get_nc()
    in_maps = _host_prep(x, weight)
    res = run_bass_kernel_spmd(nc, in_maps, core_ids=list(range(NCORES)))

    out = np.empty((16, 32, 31, 31, 31), dtype=np.float32)
    for c in range(NCORES):
        o = res.results[c]["o"]  # [nci, npr, 62, 961]
        for cl in range(NCI):
            ci = c * NCI + cl
            for pr in range(NPR):
                out[2 * pr, ci] = o[cl, pr, 0:31].reshape(31, 31, 31)
                out[2 * pr + 1, ci] = o[cl, pr, 31:62].reshape(31, 31, 31)
    return out
